# revision 11
# baseline (speedup 1.0000x reference)
"""AlignmentEncoder Trainium2 kernel (8 NeuronCores, SPMD).

Math (per batch b):
  k1   = relu(conv1d(keys, wk1, k=3, pad=1) + bk1)        (1024, 160)
  kenc = conv1d(k1, wk2, k=1) + bk2                        (80, 160)
  q1   = relu(conv1d(queries, wq1, k=3, pad=1) + bq1)      (160, 800)
  q2   = relu(conv1d(q1, wq2, k=1) + bq2)                  (80, 800)
  qenc = conv1d(q2, wq3, k=1) + bq3                        (80, 800)
  attn = -TEMP * sum_c (qenc[:,t1] - kenc[:,t2])^2         (800, 160)
  lp   = log_softmax(attn, t2) + log(prior + 1e-8)
  out  = (softmax(lp + maskbias, t2), lp)

Sharding: core c -> batch b=c//2, half h=c%2 of Tde=800.
The heavy keys-conv (wk1, 6.3MB) is split 8 ways on its 1024 output
channels: every core computes a 128-channel slice for ALL batches, a
partial kenc contribution, and a ReduceScatter (slots duplicated per
batch) hands each core the full kenc of its own batch.

The L2 distance is a matmul via (q-k)^2 = q^2 - 2qk + k^2 with
augmented operands lhsT=[qenc; 1; Q2], rhs=[2T*kenc; -T*K2; -T].
Softmax runs with t1 on partitions, t2 on the free axis; all logits
are <= 0 so no max-subtraction is needed; Exp's accum_out provides the
row sums for free.
"""

import os

import numpy as np

import concourse.bacc as bacc
import concourse.mybir as mybir
import concourse.tile as tile
from concourse.bass_utils import run_bass_kernel_spmd

N_CORES = 8
B, CQ, CK, CA = 4, 80, 512, 80
TDE, TEN = 800, 160
TENP = TEN + 2
HALF = TDE // 2          # 400 t1 positions per core
QSL = HALF + 2           # 402 queries slice width (with halo)
MT = 100                 # t1 tile size for distance/softmax
NMT = HALF // MT         # 4
NKC = CK // 128          # 4 Cin chunks for the keys conv
TEMP = np.float32(0.0005)

F32 = mybir.dt.float32
F32R = mybir.dt.float32r
BF16 = mybir.dt.bfloat16
AF = mybir.ActivationFunctionType
ALU = mybir.AluOpType

# Matmul compute dtype for the two conv paths: "f32", "f32r", or "bf16".
KDT = os.environ.get("ALENC_KDT", "f32r")
QDT = os.environ.get("ALENC_QDT", "f32r")


def _io_dt(kind):
    return {"f32": F32, "f32r": F32R, "bf16": BF16}[kind]


def build_nc(kdt=KDT, qdt=QDT, use_collective=True):
    """Build the SPMD Bass program (identical on all 8 cores)."""
    nc = bacc.Bacc(
        "TRN2", target_bir_lowering=False, debug=False, num_devices=N_CORES
    )
    kio = _io_dt(kdt)   # storage dtype of keys/wk1/wk2
    qio = _io_dt(qdt)   # storage dtype of queries/wq*

    def inp(name, shape, dt=F32):
        return nc.dram_tensor(name, shape, dt, kind="ExternalInput").ap()

    keys_pad = inp("keys_pad", [B, CK, TENP], kio)
    wk1c = inp("wk1c", [3, CK, 128], kio)
    wk2c = inp("wk2c", [128, CA], kio)
    bk1c = inp("bk1c", [128, 1])
    bk2 = inp("bk2", [CA, 1])
    qsl_d = inp("qsl", [CQ, QSL], qio)
    wq1t = inp("wq1t", [3, CQ, 2 * CQ], qio)
    wq2t = inp("wq2t", [2 * CQ, CQ], qio)
    wq3t = inp("wq3t", [CQ, CA], qio)
    bq1 = inp("bq1", [2 * CQ, 1])
    bq2 = inp("bq2", [CA, 1])
    bq3 = inp("bq3", [CA, 1])
    lprior = inp("lprior", [HALF, TEN])
    negt = inp("negt", [1, TEN])

    out_attn = nc.dram_tensor("out_attn", [HALF, TEN], F32, kind="ExternalOutput").ap()
    out_lp = nc.dram_tensor("out_lp", [HALF, TEN], F32, kind="ExternalOutput").ap()

    with tile.TileContext(nc) as tc:
        with (
            tc.tile_pool(name="sb", bufs=1) as sb,
            tc.tile_pool(name="sb2", bufs=2) as sb2,
            tc.tile_pool(name="ps", bufs=2, space="PSUM") as ps,
            tc.tile_pool(name="dram", bufs=1, space="DRAM") as dram,
        ):
            # --- ACT table warm-up: touch Exp+Ln early so the table load
            # overlaps the initial DMA phase instead of the softmax.
            scr = sb.tile([1, 1], F32, tag="scr")
            nc.gpsimd.memset(scr[:], 1.0)
            scr2 = sb.tile([1, 1], F32, tag="scr2")
            nc.scalar.activation(scr2[:], scr[:], AF.Exp)
            scr3 = sb.tile([1, 1], F32, tag="scr3")
            nc.scalar.activation(scr3[:], scr[:], AF.Ln)

            # --- small constants/biases
            bk1c_t = sb.tile([128, 1], F32, tag="bk1c")
            nc.sync.dma_start(out=bk1c_t[:], in_=bk1c[:])
            bk2_t = sb.tile([CA, 1], F32, tag="bk2")
            nc.sync.dma_start(out=bk2_t[:], in_=bk2[:])
            bq1_t = sb.tile([CQ, 2], F32, tag="bq1")
            for mh in range(2):
                nc.sync.dma_start(
                    out=bq1_t[:, mh : mh + 1], in_=bq1[mh * CQ : (mh + 1) * CQ, :]
                )
            bq2_t = sb.tile([CA, 1], F32, tag="bq2")
            nc.sync.dma_start(out=bq2_t[:], in_=bq2[:])
            bq3_t = sb.tile([CA, 1], F32, tag="bq3")
            nc.sync.dma_start(out=bq3_t[:], in_=bq3[:])
            ones80 = sb.tile([CA, 1], F32, tag="ones80")
            nc.vector.memset(ones80[:], 1.0)

            # =========== K path: conv1(k=3) over OUR 128-channel slice,
            # for all 4 batches (as 2 batch-pairs, N=320 per matmul).
            wk1s = sb.tile([128, 12 * 128], kio, tag="wk1s")
            for tap in range(3):
                for kc in range(NKC):
                    i = tap * NKC + kc
                    nc.sync.dma_start(
                        out=wk1s[:, i * 128 : (i + 1) * 128],
                        in_=wk1c[tap, kc * 128 : (kc + 1) * 128, :],
                    )
            wk2s = sb.tile([128, CA], kio, tag="wk2s")
            nc.sync.dma_start(out=wk2s[:], in_=wk2c[:])

            kp = {}
            for p in range(2):
                for kc in range(NKC):
                    t = sb.tile([128, 2 * TENP], kio, tag=f"kp{p}{kc}")
                    src = keys_pad[2 * p : 2 * p + 2, kc * 128 : (kc + 1) * 128, :]
                    nc.sync.dma_start(
                        out=t[:], in_=src.rearrange("b c t -> c b t")
                    )
                    kp[p, kc] = t

            cc_in = dram.tile([2 * B, CA, TEN], F32)
            for p in range(2):
                psk = ps.tile([128, 2 * TEN], F32, tag="big")
                n = 0
                for kc in range(NKC):
                    for tap in range(3):
                        i = tap * NKC + kc
                        rhs = kp[p, kc][:].rearrange("c (b t) -> c b t", b=2)[
                            :, :, tap : tap + TEN
                        ]
                        lhsT = wk1s[:, i * 128 : (i + 1) * 128]
                        nc.tensor.matmul(
                            psk[:].rearrange("c (b t) -> c b t", b=2),
                            lhsT,
                            rhs,
                            start=(n == 0),
                            stop=(n == 11),
                        )
                        n += 1
                k1s = sb.tile([128, 2 * TEN], kio, tag=f"k1s{p}")
                nc.vector.tensor_scalar(
                    out=k1s[:],
                    in0=psk[:],
                    scalar1=bk1c_t[:],
                    scalar2=0.0,
                    op0=ALU.add,
                    op1=ALU.max,
                )
                kep = ps.tile([CA, 2 * TEN], F32, tag="mid")
                nc.tensor.matmul(
                    kep[:],
                    wk2s[:],
                    k1s[:],
                    start=True,
                    stop=True,
                )
                kes = sb.tile([CA, 2 * TEN], F32, tag=f"kes{p}")
                nc.vector.tensor_copy(kes[:], kep[:])
                for j in range(2):
                    b = 2 * p + j
                    for dup in range(2):
                        nc.sync.dma_start(
                            out=cc_in[2 * b + dup],
                            in_=kes[:, j * TEN : (j + 1) * TEN],
                        )

            cc_out = dram.tile([CA, TEN], F32)
            if use_collective:
                nc.gpsimd.collective_compute(
                    "ReduceScatter",
                    ALU.add,
                    replica_groups=[list(range(N_CORES))],
                    ins=[cc_in.opt()],
                    outs=[cc_out.opt()],
                )
            else:
                # timing-sim variant: stand-in DMA of the same output size
                nc.sync.dma_start(out=cc_out[:], in_=cc_in[0])

            ke_raw = sb.tile([CA, TEN], F32, tag="ke_raw")
            nc.sync.dma_start(out=ke_raw[:], in_=cc_out[:])

            # distance rhs pieces: ke2 = 2*TEMP*kenc (K=80 part),
            # aux_k = [-TEMP*K2 ; -TEMP] (K=2 part).
            ke2 = sb.tile([CA, TEN], F32, tag="ke2")
            nc.vector.tensor_scalar(
                out=ke2[:],
                in0=ke_raw[:],
                scalar1=bk2_t[:],
                scalar2=float(2.0 * TEMP),
                op0=ALU.add,
                op1=ALU.mult,
            )
            ke_s = sb.tile([CA, TEN], F32, tag="ke_s")
            nc.vector.tensor_scalar_add(ke_s[:], ke_raw[:], bk2_t[:])
            ksq = sb.tile([CA, TEN], F32, tag="ksq")
            nc.vector.tensor_mul(ksq[:], ke_s[:], ke_s[:])
            k2p = ps.tile([1, TEN], F32, tag="rowp")
            nc.tensor.matmul(k2p[:], ones80[:], ksq[:], start=True, stop=True)
            aux_k = sb.tile([2, TEN], F32, tag="aux_k")
            nc.vector.tensor_scalar_mul(aux_k[0:1, :], k2p[:], float(-TEMP))
            # row 1 (base partition 1 - compute engines can't address it):
            # DMA the host-provided constant row instead.
            nc.sync.dma_start(out=aux_k[1:2, :], in_=negt[:])

            # =========== Q path (our 400-wide t1 slice)
            qsl = sb.tile([CQ, QSL], qio, tag="qsl")
            nc.sync.dma_start(out=qsl[:], in_=qsl_d[:])
            wq1s = sb.tile([CQ, 3 * 2 * CQ], qio, tag="wq1s")
            for tap in range(3):
                nc.sync.dma_start(
                    out=wq1s[:, tap * 2 * CQ : (tap + 1) * 2 * CQ], in_=wq1t[tap]
                )
            wq2s = sb.tile([CQ, 2 * CQ], qio, tag="wq2s")
            for mh in range(2):
                nc.sync.dma_start(
                    out=wq2s[:, mh * CQ : (mh + 1) * CQ],
                    in_=wq2t[mh * CQ : (mh + 1) * CQ, :],
                )
            wq3s = sb.tile([CQ, CA], qio, tag="wq3s")
            nc.sync.dma_start(out=wq3s[:], in_=wq3t[:])

            q1s = {}
            for mh in range(2):
                q1p = ps.tile([2 * CQ // 2, HALF], F32, tag="big")
                for tap in range(3):
                    lhsT = wq1s[
                        :, tap * 2 * CQ + mh * CQ : tap * 2 * CQ + mh * CQ + CQ
                    ]
                    nc.tensor.matmul(
                        q1p[:],
                        lhsT,
                        qsl[:, tap : tap + HALF],
                        start=(tap == 0),
                        stop=(tap == 2),
                    )
                t = sb.tile([CQ, HALF], qio, tag=f"q1s{mh}")
                nc.vector.tensor_scalar(
                    out=t[:],
                    in0=q1p[:],
                    scalar1=bq1_t[:, mh : mh + 1],
                    scalar2=0.0,
                    op0=ALU.add,
                    op1=ALU.max,
                )
                q1s[mh] = t

            q2p = ps.tile([CA, HALF], F32, tag="mid")
            for mh in range(2):
                nc.tensor.matmul(
                    q2p[:],
                    wq2s[:, mh * CQ : (mh + 1) * CQ],
                    q1s[mh][:],
                    start=(mh == 0),
                    stop=(mh == 1),
                )
            q2s = sb.tile([CQ, HALF], qio, tag="q2s")
            nc.vector.tensor_scalar(
                out=q2s[:],
                in0=q2p[:],
                scalar1=bq2_t[:],
                scalar2=0.0,
                op0=ALU.add,
                op1=ALU.max,
            )
            q3p = ps.tile([CA, HALF], F32, tag="mid")
            nc.tensor.matmul(
                q3p[:],
                wq3s[:],
                q2s[:],
                start=True,
                stop=True,
            )

            # distance lhs pieces: qe = qenc (K=80), aux_q = [1 ; Q2] (K=2)
            qe = sb.tile([CA, HALF], F32, tag="qe")
            nc.vector.tensor_scalar_add(qe[:], q3p[:], bq3_t[:])
            qsq = sb.tile([CA, HALF], F32, tag="qsq")
            nc.vector.tensor_mul(qsq[:], qe[:], qe[:])
            q2sum = ps.tile([1, HALF], F32, tag="rowp")
            nc.tensor.matmul(q2sum[:], ones80[:], qsq[:], start=True, stop=True)
            q2row = sb.tile([1, HALF], F32, tag="q2row")
            nc.vector.tensor_copy(q2row[:], q2sum[:])
            aux_q = sb.tile([2, HALF], F32, tag="aux_q")
            nc.vector.memset(aux_q[0:1, :], 1.0)
            nc.sync.dma_start(out=aux_q[1:2, :], in_=q2row[:])

            # =========== distance matmul + two softmaxes, 4 t1-tiles of 100
            lpr = {}
            for m in range(NMT):
                t = sb.tile([MT, TEN], F32, tag=f"lpr{m}")
                nc.sync.dma_start(out=t[:], in_=lprior[m * MT : (m + 1) * MT, :])
                lpr[m] = t

            sums = sb.tile([MT, NMT], F32, tag="sums")
            dpair = {}
            for pr in range(NMT // 2):
                dpair[pr] = ps.tile([MT, 2 * TEN], F32, tag="dist", name=f"dp{pr}")
            dps = {}
            for m in range(NMT):
                dp = dpair[m // 2][:, (m % 2) * TEN : (m % 2 + 1) * TEN]
                nc.tensor.matmul(
                    dp,
                    qe[:, m * MT : (m + 1) * MT],
                    ke2[:],
                    start=True,
                    stop=False,
                )
                nc.tensor.matmul(
                    dp,
                    aux_q[:, m * MT : (m + 1) * MT],
                    aux_k[:],
                    start=False,
                    stop=True,
                )
                dps[m] = dp
                escr = sb2.tile([MT, TEN], F32, tag="escr")
                nc.scalar.activation(
                    escr[:], dp, AF.Exp, accum_out=sums[:, m : m + 1]
                )

            logz = sb.tile([MT, NMT], F32, tag="logz")
            nc.scalar.activation(logz[:], sums[:], AF.Ln)

            for m in range(NMT):
                lp_t = sb2.tile([MT, TEN], F32, tag="lp_t")
                nc.vector.scalar_tensor_tensor(
                    out=lp_t[:],
                    in0=dps[m],
                    scalar=logz[:, m : m + 1],
                    in1=lpr[m][:],
                    op0=ALU.subtract,
                    op1=ALU.add,
                )
                nc.sync.dma_start(
                    out=out_lp[m * MT : (m + 1) * MT, :], in_=lp_t[:]
                )
                exps2 = sb2.tile([MT, TEN], F32, tag="exps2")
                ssum2 = sb2.tile([MT, 1], F32, tag="ssum2")
                nc.scalar.activation(
                    exps2[:], lp_t[:], AF.Exp, accum_out=ssum2[:]
                )
                rinv = sb2.tile([MT, 1], F32, tag="rinv")
                nc.vector.reciprocal(rinv[:], ssum2[:])
                attn_t = sb2.tile([MT, TEN], F32, tag="attn_t")
                nc.vector.tensor_scalar_mul(attn_t[:], exps2[:], rinv[:])
                nc.sync.dma_start(
                    out=out_attn[m * MT : (m + 1) * MT, :], in_=attn_t[:]
                )

    nc.compile()
    return nc


def prep_in_maps(inputs, kdt=KDT, qdt=QDT):
    """Host-side slicing/transposes -> per-core input dicts."""
    f32 = np.float32
    queries = np.asarray(inputs["queries"], f32)
    keys = np.asarray(inputs["keys"], f32)
    attn_prior = np.asarray(inputs["attn_prior"], f32)
    wk1 = np.asarray(inputs["wk1"], f32)
    bk1 = np.asarray(inputs["bk1"], f32)
    wk2 = np.asarray(inputs["wk2"], f32)
    bk2 = np.asarray(inputs["bk2"], f32)
    wq1 = np.asarray(inputs["wq1"], f32)
    bq1 = np.asarray(inputs["bq1"], f32)
    wq2 = np.asarray(inputs["wq2"], f32)
    bq2 = np.asarray(inputs["bq2"], f32)
    wq3 = np.asarray(inputs["wq3"], f32)
    bq3 = np.asarray(inputs["bq3"], f32)

    import ml_dtypes

    kio = ml_dtypes.bfloat16 if kdt == "bf16" else f32
    qio = ml_dtypes.bfloat16 if qdt == "bf16" else f32

    keys_pad = np.zeros((B, CK, TENP), f32)
    keys_pad[:, :, 1:-1] = keys
    keys_pad = np.ascontiguousarray(keys_pad.astype(kio))
    wk1T = np.ascontiguousarray(wk1.transpose(2, 1, 0).astype(kio))  # (3,512,1024)
    wk2T = np.ascontiguousarray(wk2[:, :, 0].T.astype(kio))          # (1024,80)

    qpad = np.zeros((B, CQ, TDE + 2), f32)
    qpad[:, :, 1:-1] = queries
    qpad = qpad.astype(qio)
    wq1T = np.ascontiguousarray(wq1.transpose(2, 1, 0).astype(qio))  # (3,80,160)
    wq2T = np.ascontiguousarray(wq2[:, :, 0].T.astype(qio))          # (160,80)
    wq3T = np.ascontiguousarray(wq3[:, :, 0].T.astype(qio))          # (80,80)

    logprior = np.log(attn_prior + np.float32(1e-8)).astype(f32)

    in_maps = []
    for c in range(N_CORES):
        b, h = c // 2, c % 2
        in_maps.append(
            {
                "keys_pad": keys_pad,
                "wk1c": np.ascontiguousarray(wk1T[:, :, c * 128 : (c + 1) * 128]),
                "wk2c": np.ascontiguousarray(wk2T[c * 128 : (c + 1) * 128, :]),
                "bk1c": np.ascontiguousarray(
                    bk1[c * 128 : (c + 1) * 128].reshape(128, 1)
                ),
                "bk2": bk2.reshape(CA, 1),
                "qsl": np.ascontiguousarray(qpad[b, :, h * HALF : h * HALF + QSL]),
                "wq1t": wq1T,
                "wq2t": wq2T,
                "wq3t": wq3T,
                "bq1": bq1.reshape(2 * CQ, 1),
                "bq2": bq2.reshape(CA, 1),
                "bq3": bq3.reshape(CA, 1),
                "lprior": np.ascontiguousarray(
                    logprior[b, h * HALF : (h + 1) * HALF, :]
                ),
                "negt": np.full((1, TEN), -TEMP, f32),
            }
        )
    return in_maps


def _numpy_fallback(inputs):
    """Pure-numpy reference path (used only when mask isn't all ones)."""
    f32 = np.float32

    def conv(x, w, b, pad):
        Bv, Ci, T = x.shape
        Co, _, K = w.shape
        xp = np.zeros((Bv, Ci, T + 2 * pad), f32)
        xp[:, :, pad : pad + T] = x
        y = np.zeros((Bv, Co, T), f32)
        for k in range(K):
            y += np.einsum("oi,bit->bot", w[:, :, k], xp[:, :, k : k + T])
        return y + b[None, :, None]

    q = np.asarray(inputs["queries"], f32)
    kk = np.asarray(inputs["keys"], f32)
    mask = np.asarray(inputs["mask"])
    prior = np.asarray(inputs["attn_prior"], f32)
    k1 = np.maximum(conv(kk, np.asarray(inputs["wk1"], f32), np.asarray(inputs["bk1"], f32), 1), 0)
    kenc = conv(k1, np.asarray(inputs["wk2"], f32), np.asarray(inputs["bk2"], f32), 0)
    q1 = np.maximum(conv(q, np.asarray(inputs["wq1"], f32), np.asarray(inputs["bq1"], f32), 1), 0)
    q2 = np.maximum(conv(q1, np.asarray(inputs["wq2"], f32), np.asarray(inputs["bq2"], f32), 0), 0)
    qenc = conv(q2, np.asarray(inputs["wq3"], f32), np.asarray(inputs["bq3"], f32), 0)
    d2 = (qenc[:, :, :, None] - kenc[:, :, None, :]) ** 2
    attn = (-TEMP * d2.sum(1))[:, None]                       # (B,1,Tde,Ten)
    attn = attn - np.log(np.exp(attn - attn.max(3, keepdims=True)).sum(3, keepdims=True)) - attn.max(3, keepdims=True)
    attn = attn + np.log(prior[:, None] + np.float32(1e-8))
    lp = attn.astype(f32)
    masked = np.where(mask[:, :, None, :], lp, -np.inf)
    mx = masked.max(3, keepdims=True)
    e = np.exp(masked - mx)
    sm = (e / e.sum(3, keepdims=True)).astype(f32)
    return sm, lp


_CACHE = {}


def kernel(**inputs):
    mask = np.asarray(inputs["mask"])
    if not mask.all():
        return _numpy_fallback(inputs)

    key = (KDT, QDT)
    if key not in _CACHE:
        _CACHE[key] = build_nc(kdt=KDT, qdt=QDT, use_collective=True)
    nc = _CACHE[key]

    in_maps = prep_in_maps(inputs, kdt=KDT, qdt=QDT)
    res = run_bass_kernel_spmd(nc, in_maps, list(range(N_CORES)), trace=False)

    attn = np.empty((B, 1, TDE, TEN), np.float32)
    lp = np.empty((B, 1, TDE, TEN), np.float32)
    for c in range(N_CORES):
        b, h = c // 2, c % 2
        attn[b, 0, h * HALF : (h + 1) * HALF, :] = res.results[c]["out_attn"]
        lp[b, 0, h * HALF : (h + 1) * HALF, :] = res.results[c]["out_lp"]
    return attn, lp


# revision 14
# speedup vs baseline: 1.2368x; 1.2368x over previous
"""AlignmentEncoder Trainium2 kernel (8 NeuronCores, SPMD).

Math (per batch b):
  k1   = relu(conv1d(keys, wk1, k=3, pad=1) + bk1)        (1024, 160)
  kenc = conv1d(k1, wk2, k=1) + bk2                        (80, 160)
  q1   = relu(conv1d(queries, wq1, k=3, pad=1) + bq1)      (160, 800)
  q2   = relu(conv1d(q1, wq2, k=1) + bq2)                  (80, 800)
  qenc = conv1d(q2, wq3, k=1) + bq3                        (80, 800)
  attn = -TEMP * sum_c (qenc[:,t1] - kenc[:,t2])^2         (800, 160)
  lp   = log_softmax(attn, t2) + log(prior + 1e-8)
  out  = (softmax(lp + maskbias, t2), lp)

Sharding: core c -> batch b=c//2, half h=c%2 of Tde=800.
The heavy keys-conv (wk1, 6.3MB) is split 8 ways on its 1024 output
channels: every core computes a 128-channel slice for ALL batches, a
partial kenc contribution, and a ReduceScatter (slots duplicated per
batch) hands each core the full kenc of its own batch.

The L2 distance is a matmul via (q-k)^2 = q^2 - 2qk + k^2 with
augmented operands lhsT=[qenc; 1; Q2], rhs=[2T*kenc; -T*K2; -T].
Softmax runs with t1 on partitions, t2 on the free axis; all logits
are <= 0 so no max-subtraction is needed; Exp's accum_out provides the
row sums for free.
"""

import os

import numpy as np

import concourse.bacc as bacc
import concourse.mybir as mybir
import concourse.tile as tile
from concourse.bass_utils import run_bass_kernel_spmd

N_CORES = 8
B, CQ, CK, CA = 4, 80, 512, 80
TDE, TEN = 800, 160
TENP = TEN + 2
HALF = TDE // 2          # 400 t1 positions per core
QSL = HALF + 2           # 402 queries slice width (with halo)
MT = 100                 # t1 tile size for distance/softmax
NMT = HALF // MT         # 4
NKC = CK // 128          # 4 Cin chunks for the keys conv
TEMP = np.float32(0.0005)

F32 = mybir.dt.float32
F32R = mybir.dt.float32r
BF16 = mybir.dt.bfloat16
AF = mybir.ActivationFunctionType
ALU = mybir.AluOpType

# Matmul compute dtype for the two conv paths: "f32", "f32r", or "bf16".
KDT = os.environ.get("ALENC_KDT", "f32r")
QDT = os.environ.get("ALENC_QDT", "f32r")


def _io_dt(kind):
    return {"f32": F32, "f32r": F32R, "bf16": BF16}[kind]


def build_nc(kdt=KDT, qdt=QDT, use_collective=True):
    """Build the SPMD Bass program (identical on all 8 cores)."""
    nc = bacc.Bacc(
        "TRN2", target_bir_lowering=False, debug=False, num_devices=N_CORES
    )
    kio = _io_dt(kdt)   # storage dtype of keys/wk1/wk2
    qio = _io_dt(qdt)   # storage dtype of queries/wq*

    def inp(name, shape, dt=F32):
        return nc.dram_tensor(name, shape, dt, kind="ExternalInput").ap()

    keys_pad = inp("keys_pad", [B, CK, TENP], kio)
    wk1c = inp("wk1c", [3, CK, 128], kio)
    wk2c = inp("wk2c", [128, CA], kio)
    consts = inp("consts", [128, 6])
    qsl_d = inp("qsl", [CQ, QSL], qio)
    wq1t = inp("wq1t", [3, CQ, 2 * CQ], qio)
    wq2t = inp("wq2t", [2 * CQ, CQ], qio)
    wq3t = inp("wq3t", [CQ, CA], qio)
    lprior = inp("lprior", [MT, NMT * TEN])
    negt = inp("negt", [1, TEN])

    out_attn = nc.dram_tensor(
        "out_attn", [MT, NMT * TEN], F32, kind="ExternalOutput"
    ).ap()
    out_lp = nc.dram_tensor("out_lp", [MT, NMT * TEN], F32, kind="ExternalOutput").ap()

    with tile.TileContext(nc) as tc:
        with (
            tc.tile_pool(name="sb", bufs=1) as sb,
            tc.tile_pool(name="sb2", bufs=2) as sb2,
            tc.tile_pool(name="ps", bufs=2, space="PSUM") as ps,
            tc.tile_pool(name="dram", bufs=1, space="DRAM") as dram,
        ):
            # --- ACT table warm-up: touch Exp early so the exp table load
            # overlaps the initial DMA phase instead of the softmax.
            scr = sb.tile([1, 1], F32, tag="scr")
            nc.gpsimd.memset(scr[:], 1.0)
            scr2 = sb.tile([1, 1], F32, tag="scr2")
            nc.scalar.activation(scr2[:], scr[:], AF.Exp)

            # --- packed constants: one DMA for all bias vectors
            consts_t = sb.tile([128, 6], F32, tag="consts")
            nc.sync.dma_start(out=consts_t[:], in_=consts[:])
            bk1c_ap = consts_t[:, 0:1]
            bk2_ap = consts_t[0:CA, 1:2]
            bq1_ap = [consts_t[0:CQ, 2:3], consts_t[0:CQ, 3:4]]
            bq2_ap = consts_t[0:CA, 4:5]
            bq3_ap = consts_t[0:CA, 5:6]
            ones80 = sb.tile([CA, 1], F32, tag="ones80")
            nc.vector.memset(ones80[:], 1.0)

            # =========== K path: conv1(k=3) over OUR 128-channel slice,
            # for all 4 batches (as 2 batch-pairs, N=320 per matmul).
            wk1s = sb.tile([128, 12 * 128], kio, tag="wk1s")
            nc.sync.dma_start(
                out=wk1s[:],
                in_=wk1c.rearrange("t (k c) o -> c t k o", c=128),
            )
            wk2s = sb.tile([128, CA], kio, tag="wk2s")
            nc.sync.dma_start(out=wk2s[:], in_=wk2c[:])

            kpb = {}
            for p in range(2):
                t = sb.tile([128, NKC * 2 * TENP], kio, tag=f"kpb{p}", name=f"kpb{p}")
                for j in range(2):
                    nc.sync.dma_start(
                        out=t[:].rearrange("c (k b t) -> c k b t", k=NKC, b=2)[
                            :, :, j, :
                        ],
                        in_=keys_pad[2 * p + j].rearrange("(k c) t -> c k t", c=128),
                    )
                kpb[p] = t

            cc_in = dram.tile([2 * B, CA, TEN], F32)
            for p in range(2):
                psk = ps.tile([128, 2 * TEN], F32, tag="big")
                n = 0
                for kc in range(NKC):
                    for tap in range(3):
                        i = tap * NKC + kc
                        rhs = kpb[p][:].rearrange(
                            "c (k b t) -> c k b t", k=NKC, b=2
                        )[:, kc, :, tap : tap + TEN]
                        lhsT = wk1s[:, i * 128 : (i + 1) * 128]
                        nc.tensor.matmul(
                            psk[:].rearrange("c (b t) -> c b t", b=2),
                            lhsT,
                            rhs,
                            start=(n == 0),
                            stop=(n == 11),
                        )
                        n += 1
                k1s = sb.tile([128, 2 * TEN], kio, tag=f"k1s{p}", name=f"k1s{p}")
                nc.vector.tensor_scalar(
                    out=k1s[:],
                    in0=psk[:],
                    scalar1=bk1c_ap,
                    scalar2=0.0,
                    op0=ALU.add,
                    op1=ALU.max,
                )
                kep = ps.tile([CA, 2 * TEN], F32, tag="mid")
                nc.tensor.matmul(
                    kep[:], wk2s[:], k1s[:], start=True, stop=True
                )
                # duplicate each batch block: [b0,b0,b1,b1] -> one DMA per pair
                kdup = sb.tile([CA, 4 * TEN], F32, tag=f"kdup{p}", name=f"kdup{p}")
                for j in range(2):
                    for dup in range(2):
                        nc.vector.tensor_copy(
                            kdup[:, (2 * j + dup) * TEN : (2 * j + dup + 1) * TEN],
                            kep[:, j * TEN : (j + 1) * TEN],
                        )
                nc.sync.dma_start(
                    out=cc_in[4 * p : 4 * p + 4].rearrange("s c t -> c s t"),
                    in_=kdup[:].rearrange("c (s t) -> c s t", s=4),
                )

            cc_out = dram.tile([CA, TEN], F32)
            if use_collective:
                nc.gpsimd.collective_compute(
                    "ReduceScatter",
                    ALU.add,
                    replica_groups=[list(range(N_CORES))],
                    ins=[cc_in.opt()],
                    outs=[cc_out.opt()],
                )
            else:
                # timing-sim variant: stand-in DMA of the same output size
                nc.sync.dma_start(out=cc_out[:], in_=cc_in[0])

            ke_raw = sb.tile([CA, TEN], F32, tag="ke_raw")
            nc.sync.dma_start(out=ke_raw[:], in_=cc_out[:])

            # distance rhs pieces: ke2 = 2*TEMP*kenc (K=80 part),
            # aux_k = [-TEMP*K2 ; -TEMP] (K=2 part).
            ke2 = sb.tile([CA, TEN], F32, tag="ke2")
            nc.vector.tensor_scalar(
                out=ke2[:],
                in0=ke_raw[:],
                scalar1=bk2_ap,
                scalar2=float(2.0 * TEMP),
                op0=ALU.add,
                op1=ALU.mult,
            )
            ke_s = sb.tile([CA, TEN], F32, tag="ke_s")
            nc.vector.tensor_scalar_add(ke_s[:], ke_raw[:], bk2_ap)
            ksq = sb.tile([CA, TEN], F32, tag="ksq")
            nc.vector.tensor_mul(ksq[:], ke_s[:], ke_s[:])
            k2p = ps.tile([1, TEN], F32, tag="rowp")
            nc.tensor.matmul(k2p[:], ones80[:], ksq[:], start=True, stop=True)
            aux_k = sb.tile([2, TEN], F32, tag="aux_k")
            nc.vector.tensor_scalar_mul(aux_k[0:1, :], k2p[:], float(-TEMP))
            # row 1 (base partition 1 - compute engines can't address it):
            # DMA the host-provided constant row via the SWDGE path.
            nc.gpsimd.dma_start(out=aux_k[1:2, :], in_=negt[:])

            # =========== Q path (our 400-wide t1 slice)
            qsl = sb.tile([CQ, QSL], qio, tag="qsl")
            nc.sync.dma_start(out=qsl[:], in_=qsl_d[:])
            wq1s = sb.tile([CQ, 3 * 2 * CQ], qio, tag="wq1s")
            nc.sync.dma_start(
                out=wq1s[:], in_=wq1t.rearrange("t c o -> c t o")
            )
            wq2s = sb.tile([CQ, 2 * CQ], qio, tag="wq2s")
            nc.sync.dma_start(
                out=wq2s[:], in_=wq2t.rearrange("(h c) o -> c h o", c=CQ)
            )
            wq3s = sb.tile([CQ, CA], qio, tag="wq3s")
            nc.sync.dma_start(out=wq3s[:], in_=wq3t[:])

            q1s = {}
            for mh in range(2):
                q1p = ps.tile([CQ, HALF], F32, tag="big")
                for tap in range(3):
                    lhsT = wq1s[
                        :, tap * 2 * CQ + mh * CQ : tap * 2 * CQ + mh * CQ + CQ
                    ]
                    nc.tensor.matmul(
                        q1p[:],
                        lhsT,
                        qsl[:, tap : tap + HALF],
                        start=(tap == 0),
                        stop=(tap == 2),
                    )
                t = sb.tile([CQ, HALF], qio, tag=f"q1s{mh}", name=f"q1s{mh}")
                nc.vector.tensor_scalar(
                    out=t[:],
                    in0=q1p[:],
                    scalar1=bq1_ap[mh],
                    scalar2=0.0,
                    op0=ALU.add,
                    op1=ALU.max,
                )
                q1s[mh] = t

            q2p = ps.tile([CA, HALF], F32, tag="mid")
            for mh in range(2):
                nc.tensor.matmul(
                    q2p[:],
                    wq2s[:, mh * CQ : (mh + 1) * CQ],
                    q1s[mh][:],
                    start=(mh == 0),
                    stop=(mh == 1),
                )
            q2s = sb.tile([CQ, HALF], qio, tag="q2s")
            nc.vector.tensor_scalar(
                out=q2s[:],
                in0=q2p[:],
                scalar1=bq2_ap,
                scalar2=0.0,
                op0=ALU.add,
                op1=ALU.max,
            )
            q3p = ps.tile([CA, HALF], F32, tag="mid")
            nc.tensor.matmul(q3p[:], wq3s[:], q2s[:], start=True, stop=True)

            # distance lhs pieces: qe = qenc (K=80), aux_q = [1 ; Q2] (K=2)
            qe = sb.tile([CA, HALF], F32, tag="qe")
            nc.vector.tensor_scalar_add(qe[:], q3p[:], bq3_ap)
            qsq = sb.tile([CA, HALF], F32, tag="qsq")
            nc.vector.tensor_mul(qsq[:], qe[:], qe[:])
            q2sum = ps.tile([1, HALF], F32, tag="rowp")
            nc.tensor.matmul(q2sum[:], ones80[:], qsq[:], start=True, stop=True)
            q2row = sb.tile([1, HALF], F32, tag="q2row")
            nc.vector.tensor_copy(q2row[:], q2sum[:])
            aux_q = sb.tile([2, HALF], F32, tag="aux_q")
            nc.vector.memset(aux_q[0:1, :], 1.0)
            nc.gpsimd.dma_start(out=aux_q[1:2, :], in_=q2row[:])

            # =========== distance matmul + two softmaxes, 4 t1-tiles of 100.
            # log_softmax(x) + lprior = (x + lprior) - log(sum exp x); the
            # second softmax of that is softmax(x + lprior) (logZ cancels),
            # so ALL Exps run before the single Ln -> one table switch.
            lpr_t = sb.tile([MT, NMT * TEN], F32, tag="lpr_t")
            nc.sync.dma_start(out=lpr_t[:], in_=lprior[:])

            sums = sb.tile([MT, NMT], F32, tag="sums")
            ssum2 = sb.tile([MT, NMT], F32, tag="ssum2")
            attn_all = sb.tile([MT, NMT * TEN], F32, tag="attn_all")
            lp_all = sb.tile([MT, NMT * TEN], F32, tag="lp_all")
            xlp = {}
            e2 = {}
            dpair = {}
            for pr in range(NMT // 2):
                dpair[pr] = ps.tile([MT, 2 * TEN], F32, tag="dist", name=f"dp{pr}")
            for m in range(NMT):
                dp = dpair[m // 2][:, (m % 2) * TEN : (m % 2 + 1) * TEN]
                nc.tensor.matmul(
                    dp,
                    qe[:, m * MT : (m + 1) * MT],
                    ke2[:],
                    start=True,
                    stop=False,
                )
                nc.tensor.matmul(
                    dp,
                    aux_q[:, m * MT : (m + 1) * MT],
                    aux_k[:],
                    start=False,
                    stop=True,
                )
                escr = sb2.tile([MT, TEN], F32, tag="escr")
                nc.scalar.activation(
                    escr[:], dp, AF.Exp, accum_out=sums[:, m : m + 1]
                )
                x = sb.tile([MT, TEN], F32, tag=f"xlp{m}", name=f"xlp{m}")
                nc.vector.tensor_add(
                    x[:], dp, lpr_t[:, m * TEN : (m + 1) * TEN]
                )
                xlp[m] = x
                e = sb.tile([MT, TEN], F32, tag=f"e2{m}", name=f"e2{m}")
                nc.scalar.activation(
                    e[:], x[:], AF.Exp, accum_out=ssum2[:, m : m + 1]
                )
                e2[m] = e

            logz = sb.tile([MT, NMT], F32, tag="logz")
            nc.scalar.activation(logz[:], sums[:], AF.Ln)
            rinv = sb.tile([MT, NMT], F32, tag="rinv")
            nc.vector.reciprocal(rinv[:], ssum2[:])

            for m in range(NMT):
                nc.vector.tensor_scalar_mul(
                    attn_all[:, m * TEN : (m + 1) * TEN],
                    e2[m][:],
                    rinv[:, m : m + 1],
                )
                nc.vector.tensor_scalar_sub(
                    lp_all[:, m * TEN : (m + 1) * TEN],
                    xlp[m][:],
                    logz[:, m : m + 1],
                )
            nc.gpsimd.dma_start(out=out_attn[:], in_=attn_all[:])
            nc.gpsimd.dma_start(out=out_lp[:], in_=lp_all[:])

    nc.compile()
    return nc


def prep_in_maps(inputs, kdt=KDT, qdt=QDT):
    """Host-side slicing/transposes -> per-core input dicts."""
    f32 = np.float32
    queries = np.asarray(inputs["queries"], f32)
    keys = np.asarray(inputs["keys"], f32)
    attn_prior = np.asarray(inputs["attn_prior"], f32)
    wk1 = np.asarray(inputs["wk1"], f32)
    bk1 = np.asarray(inputs["bk1"], f32)
    wk2 = np.asarray(inputs["wk2"], f32)
    bk2 = np.asarray(inputs["bk2"], f32)
    wq1 = np.asarray(inputs["wq1"], f32)
    bq1 = np.asarray(inputs["bq1"], f32)
    wq2 = np.asarray(inputs["wq2"], f32)
    bq2 = np.asarray(inputs["bq2"], f32)
    wq3 = np.asarray(inputs["wq3"], f32)
    bq3 = np.asarray(inputs["bq3"], f32)

    import ml_dtypes

    kio = ml_dtypes.bfloat16 if kdt == "bf16" else f32
    qio = ml_dtypes.bfloat16 if qdt == "bf16" else f32

    keys_pad = np.zeros((B, CK, TENP), f32)
    keys_pad[:, :, 1:-1] = keys
    keys_pad = np.ascontiguousarray(keys_pad.astype(kio))
    wk1T = np.ascontiguousarray(wk1.transpose(2, 1, 0).astype(kio))  # (3,512,1024)
    wk2T = np.ascontiguousarray(wk2[:, :, 0].T.astype(kio))          # (1024,80)

    qpad = np.zeros((B, CQ, TDE + 2), f32)
    qpad[:, :, 1:-1] = queries
    qpad = qpad.astype(qio)
    wq1T = np.ascontiguousarray(wq1.transpose(2, 1, 0).astype(qio))  # (3,80,160)
    wq2T = np.ascontiguousarray(wq2[:, :, 0].T.astype(qio))          # (160,80)
    wq3T = np.ascontiguousarray(wq3[:, :, 0].T.astype(qio))          # (80,80)

    logprior = np.log(attn_prior + np.float32(1e-8)).astype(f32)

    in_maps = []
    for c in range(N_CORES):
        b, h = c // 2, c % 2
        consts = np.zeros((128, 6), f32)
        consts[:, 0] = bk1[c * 128 : (c + 1) * 128]
        consts[:CA, 1] = bk2
        consts[:CQ, 2] = bq1[0:CQ]
        consts[:CQ, 3] = bq1[CQ : 2 * CQ]
        consts[:CA, 4] = bq2
        consts[:CA, 5] = bq3
        lp_slice = logprior[b, h * HALF : (h + 1) * HALF, :]
        lp_il = np.ascontiguousarray(
            lp_slice.reshape(NMT, MT, TEN).transpose(1, 0, 2).reshape(MT, NMT * TEN)
        )
        in_maps.append(
            {
                "keys_pad": keys_pad,
                "wk1c": np.ascontiguousarray(wk1T[:, :, c * 128 : (c + 1) * 128]),
                "wk2c": np.ascontiguousarray(wk2T[c * 128 : (c + 1) * 128, :]),
                "consts": consts,
                "qsl": np.ascontiguousarray(qpad[b, :, h * HALF : h * HALF + QSL]),
                "wq1t": wq1T,
                "wq2t": wq2T,
                "wq3t": wq3T,
                "lprior": lp_il,
                "negt": np.full((1, TEN), -TEMP, f32),
            }
        )
    return in_maps


def _numpy_fallback(inputs):
    """Pure-numpy reference path (used only when mask isn't all ones)."""
    f32 = np.float32

    def conv(x, w, b, pad):
        Bv, Ci, T = x.shape
        Co, _, K = w.shape
        xp = np.zeros((Bv, Ci, T + 2 * pad), f32)
        xp[:, :, pad : pad + T] = x
        y = np.zeros((Bv, Co, T), f32)
        for k in range(K):
            y += np.einsum("oi,bit->bot", w[:, :, k], xp[:, :, k : k + T])
        return y + b[None, :, None]

    q = np.asarray(inputs["queries"], f32)
    kk = np.asarray(inputs["keys"], f32)
    mask = np.asarray(inputs["mask"])
    prior = np.asarray(inputs["attn_prior"], f32)
    k1 = np.maximum(conv(kk, np.asarray(inputs["wk1"], f32), np.asarray(inputs["bk1"], f32), 1), 0)
    kenc = conv(k1, np.asarray(inputs["wk2"], f32), np.asarray(inputs["bk2"], f32), 0)
    q1 = np.maximum(conv(q, np.asarray(inputs["wq1"], f32), np.asarray(inputs["bq1"], f32), 1), 0)
    q2 = np.maximum(conv(q1, np.asarray(inputs["wq2"], f32), np.asarray(inputs["bq2"], f32), 0), 0)
    qenc = conv(q2, np.asarray(inputs["wq3"], f32), np.asarray(inputs["bq3"], f32), 0)
    d2 = (qenc[:, :, :, None] - kenc[:, :, None, :]) ** 2
    attn = (-TEMP * d2.sum(1))[:, None]                       # (B,1,Tde,Ten)
    attn = attn - np.log(np.exp(attn - attn.max(3, keepdims=True)).sum(3, keepdims=True)) - attn.max(3, keepdims=True)
    attn = attn + np.log(prior[:, None] + np.float32(1e-8))
    lp = attn.astype(f32)
    masked = np.where(mask[:, :, None, :], lp, -np.inf)
    mx = masked.max(3, keepdims=True)
    e = np.exp(masked - mx)
    sm = (e / e.sum(3, keepdims=True)).astype(f32)
    return sm, lp


_CACHE = {}


def kernel(**inputs):
    mask = np.asarray(inputs["mask"])
    if not mask.all():
        return _numpy_fallback(inputs)

    key = (KDT, QDT)
    if key not in _CACHE:
        _CACHE[key] = build_nc(kdt=KDT, qdt=QDT, use_collective=True)
    nc = _CACHE[key]

    in_maps = prep_in_maps(inputs, kdt=KDT, qdt=QDT)
    res = run_bass_kernel_spmd(nc, in_maps, list(range(N_CORES)), trace=False)

    attn = np.empty((B, 1, TDE, TEN), np.float32)
    lp = np.empty((B, 1, TDE, TEN), np.float32)

    def deil(r):
        return r.reshape(MT, NMT, TEN).transpose(1, 0, 2).reshape(HALF, TEN)

    for c in range(N_CORES):
        b, h = c // 2, c % 2
        attn[b, 0, h * HALF : (h + 1) * HALF, :] = deil(res.results[c]["out_attn"])
        lp[b, 0, h * HALF : (h + 1) * HALF, :] = deil(res.results[c]["out_lp"])
    return attn, lp
